# revision 2
# baseline (speedup 1.0000x reference)
"""Trainium2 Bass kernel v2: 2-layer GRU (T=512, B=128, IN=64, H=1024) +
time-distributed linear (OUT=64) on 8 NeuronCores.

v2 over baseline:
- Recurrent + gi1 + linear matmuls in fp8e4m3 with DoubleRow perf mode
  (2 K-tiles per pass -> half the PE streaming cost).
- gi / b_hh-n adds folded into the PE as identity accumulate-matmuls into
  the same PSUM group (removes 6 of 16 DVE ops per step).
- ACT reads gates straight from PSUM; elementwise chain in bf16 (2x DVE);
  h-update sub on gpsimd; transposes write one shared psum tile per half
  so a single ACT copy refreshes 4 K-tiles of hT.

Sharding: data-parallel over batch (16 per core), weights replicated.
"""

import sys

for _p in ("/opt/trn_rl_repo", "/root/.axon_site/_ro/trn_rl_repo"):
    if _p not in sys.path:
        sys.path.insert(0, _p)

import numpy as np
import ml_dtypes

import concourse.bass as bass
import concourse.mybir as mybir
import concourse.tile as tile
from concourse.bass import ds
from concourse.bass_utils import run_bass_kernel_spmd

F32 = mybir.dt.float32
BF16 = mybir.dt.bfloat16
F8 = mybir.dt.float8e4
AF = mybir.ActivationFunctionType
DR = mybir.MatmulPerfMode.DoubleRow

N_CORES = 8
B, IN, H, OUT = 128, 64, 1024, 64
BL = B // N_CORES          # 16
G3 = 3 * H                 # 3072
HALF = H // 2              # 512
NCH = G3 // 512            # 6
KT = H // 128              # 8
KP = KT // 2               # 4 double-row K-pairs


# ---- walrus workaround: split the TileContext closing drain's waits ----
def _patched_drain_and_barrier(self, tick_clock, wait_clock):
    from concourse.vector_clock import ScopedClock
    drain_inst = self.nc.sync.drain()
    wait_clock.add_sem_waits(
        drain_inst.ins, ScopedClock({None: tick_clock.global_clock}))
    mi = drain_inst.ins
    si = mi.sync_info
    waits = list(si.on_wait) if (si is not None and si.on_wait) else []
    if len(waits) > 1:
        si.on_wait = waits[:1]
        mi.sync_info = si
        for w in waits[1:]:
            extra = self.nc.sync.drain()
            emi = extra.ins
            esi = emi.sync_info
            if esi is None:
                esi = mybir.SyncInfo(on_wait=[], on_update=[])
            esi.on_wait = [w]
            emi.sync_info = esi
    self.nc.all_engine_barrier()
    assert self.sems is not None
    popped = self.nc._tile_sem_poison_stack.pop()
    assert popped is self._sem_poison
    self.nc.clear_and_free_semaphores(list(self.sems.allocated().values()))
    self.nc.all_engine_barrier()


tile.TileContext._drain_and_barrier = _patched_drain_and_barrier


def _perm_rows():
    r = np.arange(0, H)
    z = np.arange(H, 2 * H)
    n = np.arange(2 * H, 3 * H)
    return np.concatenate([
        r[:HALF], z[:HALF], n[:HALF], r[HALF:], z[HALF:], n[HALF:]])


PERM = _perm_rows()


def _build_program(T):
    NTOK = T * BL
    nc = bass.Bass("TRN2", target_bir_lowering=False, debug=False)

    xT = nc.declare_dram_parameter("xT", [IN, NTOK + 128], BF16, isOutput=False)
    w_ih0T = nc.declare_dram_parameter("w_ih0T", [IN, G3], BF16, isOutput=False)
    w_hh0T = nc.declare_dram_parameter("w_hh0T", [KT, 128, G3], F8, isOutput=False)
    w_ih1T = nc.declare_dram_parameter("w_ih1T", [KT, 128, G3], BF16, isOutput=False)
    w_hh1T = nc.declare_dram_parameter("w_hh1T", [KT, 128, G3], F8, isOutput=False)
    w_linT = nc.declare_dram_parameter("w_linT", [KT, 128, OUT], BF16, isOutput=False)
    gbias0 = nc.declare_dram_parameter("gbias0", [128, G3], F32, isOutput=False)
    gbias1 = nc.declare_dram_parameter("gbias1", [128, G3], F32, isOutput=False)
    bhn0d = nc.declare_dram_parameter("bhn0", [BL, H], BF16, isOutput=False)
    bhn1d = nc.declare_dram_parameter("bhn1", [BL, H], BF16, isOutput=False)
    identd = nc.declare_dram_parameter("identB", [BL, BL], BF16, isOutput=False)
    blind = nc.declare_dram_parameter("blin", [128, OUT], F32, isOutput=False)
    y = nc.declare_dram_parameter("y", [NTOK, OUT], F32, isOutput=True)

    with tile.TileContext(nc) as tc:
        with tc.tile_pool(name="dram", bufs=1, space="DRAM") as dpool:
            gi0 = dpool.tile([T + 2, BL, G3], BF16, tag="gi0", name="gi0")
            gi1 = dpool.tile([T + 2, BL, G3], BF16, tag="gi1", name="gi1")
            h1T = dpool.tile([T, KT, 128, BL], BF16, tag="h1T", name="h1T")
            h2T = dpool.tile([T, KT, 128, BL], BF16, tag="h2T", name="h2T")

            # ---------------- phase 1: gi0 ----------------
            with tc.tile_pool(name="p1", bufs=1) as cp, \
                 tc.tile_pool(name="p1w", bufs=3) as wp, \
                 tc.tile_pool(name="p1ps", bufs=3, space="PSUM") as pp:
                wih0 = cp.tile([IN, G3], BF16, tag="wih0", name="wih0")
                nc.sync.dma_start(out=wih0[:, :], in_=w_ih0T[:, :])
                gb = cp.tile([128, G3], F32, tag="gb0", name="gb0")
                nc.sync.dma_start(out=gb[:, :], in_=gbias0[:, :])
                for it in range(NTOK // 128):
                    xt = wp.tile([IN, 128], BF16, tag="xt", name="xt")
                    nc.sync.dma_start(
                        out=xt[:, :], in_=xT[:, it * 128:(it + 1) * 128])
                    gsb = wp.tile([128, G3], BF16, tag="gsb", name="gsb")
                    for c in range(NCH):
                        gps = pp.tile([128, 512], F32, tag="gps", name="gps")
                        nc.tensor.matmul(
                            gps[:, :], xt[:, :], wih0[:, c * 512:(c + 1) * 512],
                            start=True, stop=True)
                        nc.vector.tensor_add(
                            gsb[:, c * 512:(c + 1) * 512], gps[:, :],
                            gb[:, c * 512:(c + 1) * 512])
                    nc.sync.dma_start(
                        out=gi0[:, :, :].rearrange("t b n -> (t b) n")[
                            it * 128:(it + 1) * 128, :],
                        in_=gsb[:, :])

            # ---------------- phase 2: scan layer 0 ----------------
            _scan_layer(nc, tc, T, w_hh0T, bhn0d, identd, gi0, h1T,
                        out_bf16=True)

            # ---------------- phase 3: gi1 ----------------
            with tc.tile_pool(name="p3", bufs=1) as cp, \
                 tc.tile_pool(name="p3w", bufs=3) as wp, \
                 tc.tile_pool(name="p3g", bufs=2) as gp, \
                 tc.tile_pool(name="p3ps", bufs=3, space="PSUM") as pp:
                wih1 = cp.tile([128, KT, G3], BF16, tag="wih1", name="wih1")
                nc.sync.dma_start(
                    out=wih1[:, :, :],
                    in_=w_ih1T[:, :, :].rearrange("k p n -> p k n"))
                gb = cp.tile([128, G3], F32, tag="gb1", name="gb1")
                nc.sync.dma_start(out=gb[:, :], in_=gbias1[:, :])
                TT = 128 // BL  # timesteps per token tile = 8
                for it in range(NTOK // 128):
                    hts = wp.tile([128, KT, 128], BF16, tag="hts", name="hts")
                    for k in range(KT):
                        nc.sync.dma_start(
                            out=hts[:, k, :].rearrange("p (t b) -> p t b", t=TT),
                            in_=h1T[it * TT:(it + 1) * TT, k, :, :].rearrange(
                                "t p b -> p t b"))
                    gsb = gp.tile([128, G3], BF16, tag="gsb1", name="gsb1")
                    for c in range(NCH):
                        gps = pp.tile([128, 512], F32, tag="gps1", name="gps1")
                        for k in range(KT):
                            nc.tensor.matmul(
                                gps[:, :], hts[:, k, :],
                                wih1[:, k, c * 512:(c + 1) * 512],
                                start=(k == 0), stop=(k == KT - 1))
                        nc.vector.tensor_add(
                            gsb[:, c * 512:(c + 1) * 512], gps[:, :],
                            gb[:, c * 512:(c + 1) * 512])
                    nc.sync.dma_start(
                        out=gi1[:, :, :].rearrange("t b n -> (t b) n")[
                            it * 128:(it + 1) * 128, :],
                        in_=gsb[:, :])

            # ---------------- phase 4: scan layer 1 ----------------
            _scan_layer(nc, tc, T, w_hh1T, bhn1d, identd, gi1, h2T,
                        out_bf16=True)

            # ---------------- phase 5: linear ----------------
            with tc.tile_pool(name="p5", bufs=1) as cp, \
                 tc.tile_pool(name="p5w", bufs=3) as wp, \
                 tc.tile_pool(name="p5ps", bufs=3, space="PSUM") as pp:
                wlin = cp.tile([128, KT, OUT], BF16, tag="wlin", name="wlin")
                nc.sync.dma_start(
                    out=wlin[:, :, :],
                    in_=w_linT[:, :, :].rearrange("k p n -> p k n"))
                bl = cp.tile([128, OUT], F32, tag="bl", name="bl")
                nc.sync.dma_start(out=bl[:, :], in_=blind[:, :])
                TT = 128 // BL
                for it in range(NTOK // 128):
                    hts = wp.tile([128, KT, 128], BF16, tag="hts5", name="hts5")
                    for k in range(KT):
                        nc.sync.dma_start(
                            out=hts[:, k, :].rearrange("p (t b) -> p t b", t=TT),
                            in_=h2T[it * TT:(it + 1) * TT, k, :, :].rearrange(
                                "t p b -> p t b"))
                    ops = pp.tile([128, OUT], F32, tag="lps", name="lps")
                    for k in range(KT):
                        nc.tensor.matmul(
                            ops[:, :], hts[:, k, :], wlin[:, k, :],
                            start=(k == 0), stop=(k == KT - 1))
                    osb = wp.tile([128, OUT], F32, tag="osb", name="osb")
                    nc.vector.tensor_add(osb[:, :], ops[:, :], bl[:, :])
                    nc.sync.dma_start(
                        out=y[it * 128:(it + 1) * 128, :], in_=osb[:, :])
    _split_excess_waits(nc)
    return nc


_WAIT_LIMIT = 1


def _split_excess_waits(nc):
    """walrus CoreV3 allows only ~2 sync waits per instruction; hoist the
    excess onto NoOp instructions inserted just before, on the same engine."""
    for bb in nc.main_func.blocks:
        insts = list(bb.instructions)
        out, n_extra = [], 0
        for inst in insts:
            si = inst.sync_info
            waits = list(si.on_wait) if (si is not None and si.on_wait) else []
            if len(waits) > _WAIT_LIMIT:
                keep = waits[-_WAIT_LIMIT:]
                excess = waits[:-_WAIT_LIMIT]
                for j in range(0, len(excess), _WAIT_LIMIT):
                    nop = mybir.InstNoOp(
                        name=f"{inst.name}-w{j}-{n_extra}", ins=[], outs=[])
                    nop.engine = inst.engine
                    nop.sync_info = mybir.SyncInfo(
                        on_wait=excess[j:j + _WAIT_LIMIT], on_update=[])
                    out.append(nop)
                    n_extra += 1
                si.on_wait = keep
                inst.sync_info = si
            out.append(inst)
        if n_extra:
            bb.instructions = out
    mx = max((len(i.sync_info.on_wait) if i.sync_info and i.sync_info.on_wait else 0)
             for bb in nc.main_func.blocks for i in bb.instructions)
    print("[split_waits] max on_wait after pass:", mx)


def _scan_layer(nc, tc, T, w_hhT_dram, bhn_dram, ident_dram, gi_dram, hT_dram,
                out_bf16=False):
    with tc.tile_pool(name="scst", bufs=1) as st, \
         tc.tile_pool(name="scps", bufs=1, space="PSUM") as pp:
        whh = st.tile([128, KT, G3], F8, tag="whh", name="whh")
        nc.sync.dma_start(
            out=whh[:, :, :],
            in_=w_hhT_dram[:, :, :].rearrange("k p n -> p k n"))
        bhn = st.tile([BL, H], BF16, tag="bhn", name="bhn")
        nc.sync.dma_start(out=bhn[:, :], in_=bhn_dram[:, :])
        ident = st.tile([BL, BL], BF16, tag="ident", name="ident")
        nc.sync.dma_start(out=ident[:, :], in_=ident_dram[:, :])

        hT = [st.tile([128, KT, BL], F8, tag=f"hT{i}", name=f"hT{i}") for i in range(2)]
        hB = ([st.tile([128, KT, BL], BF16, tag=f"hB{i}", name=f"hB{i}")
               for i in range(2)] if out_bf16 else None)
        hF = [st.tile([BL, H], BF16, tag=f"hF{i}", name=f"hF{i}") for i in range(2)]
        giA = [st.tile([BL, G3], BF16, tag=f"gi{i}", name=f"gi{i}") for i in range(3)]
        gps = [pp.tile([BL, 512], F32, tag=f"gps{c}", name=f"gps{c}") for c in range(NCH)]
        tps = [pp.tile([128, 4 * BL], BF16, tag=f"tps{i}", name=f"tps{i}") for i in range(2)]
        rga = [st.tile([BL, HALF], BF16, tag=f"rg{i}", name=f"rg{i}") for i in range(2)]
        tra = [st.tile([BL, HALF], BF16, tag=f"tra{i}", name=f"tra{i}") for i in range(2)]
        tza = [st.tile([BL, HALF], BF16, tag=f"tza{i}", name=f"tza{i}") for i in range(2)]
        zga = [st.tile([BL, HALF], BF16, tag=f"zg{i}", name=f"zg{i}") for i in range(2)]
        nga = [st.tile([BL, HALF], BF16, tag=f"ng{i}", name=f"ng{i}") for i in range(2)]
        tmp = [st.tile([BL, HALF], BF16, tag=f"tmp{i}", name=f"tmp{i}") for i in range(2)]

        nc.vector.memset(hT[0][:, :, :], 0.0)
        nc.vector.memset(hF[0][:, :], 0.0)
        for s in range(2):
            nc.sync.dma_start(
                out=giA[s][:, :],
                in_=gi_dram[s:s + 1, :, :].rearrange("o b n -> (o b) n"))

        def step(t_reg, toff, par):
            hin, hout = hT[par], hT[1 - par]
            hfin, hfout = hF[par], hF[1 - par]
            gi = giA[toff % 3]

            # PE, c-major so each chunk's psum closes as early as possible
            # (half-0 elementwise overlaps half-1 matmuls). Each chunk opens
            # with an identity matmul folding gi (r/z) or b_hh-n (n) into
            # psum, then 4 fp8 DoubleRow passes (2 K-tiles each, ~2x bf16).
            for c in range(NCH):
                half, j = c // 3, c % 3
                if j < 2:
                    rhs = gi[:, c * 512:(c + 1) * 512]
                else:
                    rhs = bhn[:, half * HALF:(half + 1) * HALF]
                nc.tensor.matmul(gps[c][:, :], ident[:, :], rhs,
                                 start=True, stop=False)
                for kp in range(KP):
                    nc.tensor.matmul(
                        gps[c][:, :],
                        hin[:, 2 * kp:2 * kp + 2, :],
                        whh[:, 2 * kp:2 * kp + 2, c * 512:(c + 1) * 512],
                        start=False, stop=(kp == KP - 1), perf_mode=DR)

            for half in range(2):
                cr, cz, cn = 3 * half, 3 * half + 1, 3 * half + 2
                go = half * 3 * 512
                hs = slice(half * HALF, (half + 1) * HALF)
                nc.scalar.activation(rga[half][:, :], gps[cr][:, :], AF.Sigmoid)
                nc.scalar.activation(zga[half][:, :], gps[cz][:, :], AF.Sigmoid)
                nc.vector.tensor_mul(tmp[half][:, :], rga[half][:, :],
                                     gps[cn][:, :])
                nc.vector.tensor_add(tmp[half][:, :], tmp[half][:, :],
                                     gi[:, go + 1024:go + 1536])
                nc.scalar.activation(nga[half][:, :], tmp[half][:, :], AF.Tanh)
                nc.gpsimd.tensor_sub(tmp[half][:, :], hfin[:, hs],
                                     nga[half][:, :])
                nc.vector.tensor_mul(tmp[half][:, :], zga[half][:, :],
                                     tmp[half][:, :])
                nc.vector.tensor_add(hfout[:, hs], nga[half][:, :],
                                     tmp[half][:, :])
                for kk in range(4):
                    k = half * 4 + kk
                    nc.tensor.transpose(
                        tps[half][:, kk * BL:(kk + 1) * BL],
                        hfout[:, k * 128:(k + 1) * 128],
                        ident[:, :])
                nc.scalar.activation(
                    hout[:, half * 4:(half + 1) * 4, :].rearrange(
                        "p k b -> p (k b)"),
                    tps[half][:, :], AF.Copy)
                if out_bf16:
                    nc.vector.tensor_copy(
                        hB[1 - par][:, half * 4:(half + 1) * 4, :].rearrange(
                            "p k b -> p (k b)"),
                        tps[half][:, :])

            nc.sync.dma_start(
                out=hT_dram[ds(t_reg + toff, 1), :, :, :].rearrange(
                    "o k p b -> p (o k) b"),
                in_=(hB[1 - par] if out_bf16 else hout)[:, :, :])
            nc.sync.dma_start(
                out=giA[toff % 3][:, :],
                in_=gi_dram[ds(t_reg + toff + 2, 1), :, :].rearrange(
                    "o b n -> (o b) n"))

        with tc.For_i(0, T, 2) as t:
            step(t, 0, 0)
            step(t, 1, 1)


# ---------------- host-side wrapper ----------------
NP8 = ml_dtypes.float8_e4m3fn


def _prep_inputs(x, w_ih0, w_hh0, b_ih0, b_hh0, w_ih1, w_hh1, b_ih1, b_hh1,
                 w_lin, b_lin, T):
    NTOK = T * BL
    bf = ml_dtypes.bfloat16

    def prep_layer(w_ih, w_hh, b_ih, b_hh, ih_fp8):
        w_ihP = np.asarray(w_ih)[PERM, :]          # [3H, in]
        w_hhP = np.asarray(w_hh)[PERM, :]          # [3H, H]
        b_ihP = np.asarray(b_ih)[PERM]
        b_hhP = np.asarray(b_hh)[PERM]
        # gi bias: b_ih everywhere + b_hh on r/z blocks (n gets b_hh inside r*())
        gb = b_ihP.copy()
        for blk in range(2):
            o = blk * 3 * 512
            gb[o:o + 1024] += b_hhP[o:o + 1024]    # r and z blocks
        # b_hh n-gate halves -> [H] = [n0 n1]
        bhn = np.concatenate([b_hhP[1024:1536], b_hhP[1024 + 1536:1536 + 1536]])
        ih_dt = bf
        w_ihT = np.ascontiguousarray(w_ihP.T).astype(ih_dt)   # [in, 3H]
        w_hhT = np.ascontiguousarray(w_hhP.T).astype(NP8)     # [H, 3H]
        w_hhT = w_hhT.reshape(KT, 128, G3)
        gbB = np.broadcast_to(gb.astype(np.float32), (128, G3)).copy()
        bhnB = np.broadcast_to(bhn.astype(bf), (BL, H)).copy()
        return w_ihT, w_hhT, gbB, bhnB

    w_ih0T, w_hh0T, gb0, bhn0 = prep_layer(w_ih0, w_hh0, b_ih0, b_hh0, False)
    w_ih1T, w_hh1T, gb1, bhn1 = prep_layer(w_ih1, w_hh1, b_ih1, b_hh1, True)
    w_ih1T = w_ih1T.reshape(KT, 128, G3)
    w_linT = np.ascontiguousarray(np.asarray(w_lin).T).astype(bf).reshape(
        KT, 128, OUT)
    blinB = np.broadcast_to(np.asarray(b_lin).astype(np.float32),
                            (128, OUT)).copy()
    identB = np.eye(BL, dtype=bf)

    common = dict(w_ih0T=w_ih0T, w_hh0T=w_hh0T, w_ih1T=w_ih1T, w_hh1T=w_hh1T,
                  w_linT=w_linT, gbias0=gb0, gbias1=gb1, bhn0=bhn0, bhn1=bhn1,
                  blin=blinB, identB=identB)

    in_maps = []
    x = np.asarray(x)
    for c in range(N_CORES):
        xs = x[:T, c * BL:(c + 1) * BL, :]          # [T, BL, IN]
        xT = np.zeros((IN, NTOK + 128), dtype=bf)
        xT[:, :NTOK] = xs.reshape(NTOK, IN).T.astype(bf)
        m = dict(common)
        m["xT"] = xT
        in_maps.append(m)
    return in_maps


_NC_CACHE = {}


def get_program(T):
    if T not in _NC_CACHE:
        _NC_CACHE[T] = _build_program(T)
    return _NC_CACHE[T]


def run(x, w_ih0, w_hh0, b_ih0, b_hh0, w_ih1, w_hh1, b_ih1, b_hh1,
        w_lin, b_lin, T=512, trace=False):
    in_maps = _prep_inputs(x, w_ih0, w_hh0, b_ih0, b_hh0, w_ih1, w_hh1,
                           b_ih1, b_hh1, w_lin, b_lin, T)
    nc = get_program(T)
    res = run_bass_kernel_spmd(nc, in_maps, core_ids=list(range(N_CORES)),
                               trace=trace)
    NTOK = T * BL
    out = np.empty((T, B, OUT), dtype=np.float32)
    for c in range(N_CORES):
        out[:, c * BL:(c + 1) * BL, :] = res.results[c]["y"].reshape(
            T, BL, OUT)
    return out, res


def kernel(**inputs):
    out, _ = run(**inputs)
    return out


# revision 3
# speedup vs baseline: 1.1760x; 1.1760x over previous
"""Trainium2 Bass kernel v2: 2-layer GRU (T=512, B=128, IN=64, H=1024) +
time-distributed linear (OUT=64) on 8 NeuronCores.

Over the original baseline (29.6ms -> 17.1ms):
- Recurrent matmuls in fp8e4m3 DoubleRow perf mode (2 K-tiles per pass,
  ~2x bf16 PE throughput, measured 225ns vs 441ns per K=256xN=512 unit).
  The h state stays bf16; only the transposed matmul operand is fp8.
  h1/h2 written to DRAM in bf16 so gi1 and the linear layer stay
  full-precision (fp8 there pushes rel_err past the 2e-2 gate).
- gi / b_hh-n adds folded into the PE as identity accumulate-matmuls into
  the same PSUM group (removes 6 of 16 DVE ops per step); chunks emitted
  c-major so each psum chunk closes early and the elementwise chain of
  hidden-half 0 overlaps the half-1 matmuls.
- ACT reads gates straight from PSUM; elementwise chain in bf16 (2x DVE);
  h-update sub on gpsimd; transposes write one shared psum tile per half
  so a single ACT copy refreshes 4 K-tiles of hT.

Sharding: data-parallel over batch (16 per core), weights replicated.
"""

import sys

for _p in ("/opt/trn_rl_repo", "/root/.axon_site/_ro/trn_rl_repo"):
    if _p not in sys.path:
        sys.path.insert(0, _p)

import numpy as np
import ml_dtypes

import concourse.bass as bass
import concourse.mybir as mybir
import concourse.tile as tile
from concourse.bass import ds
from concourse.bass_utils import run_bass_kernel_spmd

F32 = mybir.dt.float32
BF16 = mybir.dt.bfloat16
F8 = mybir.dt.float8e4
AF = mybir.ActivationFunctionType
DR = mybir.MatmulPerfMode.DoubleRow

N_CORES = 8
B, IN, H, OUT = 128, 64, 1024, 64
BL = B // N_CORES          # 16
G3 = 3 * H                 # 3072
HALF = H // 2              # 512
NCH = G3 // 512            # 6
KT = H // 128              # 8
KP = KT // 2               # 4 double-row K-pairs


# ---- walrus workaround: split the TileContext closing drain's waits ----
def _patched_drain_and_barrier(self, tick_clock, wait_clock):
    from concourse.vector_clock import ScopedClock
    drain_inst = self.nc.sync.drain()
    wait_clock.add_sem_waits(
        drain_inst.ins, ScopedClock({None: tick_clock.global_clock}))
    mi = drain_inst.ins
    si = mi.sync_info
    waits = list(si.on_wait) if (si is not None and si.on_wait) else []
    if len(waits) > 1:
        si.on_wait = waits[:1]
        mi.sync_info = si
        for w in waits[1:]:
            extra = self.nc.sync.drain()
            emi = extra.ins
            esi = emi.sync_info
            if esi is None:
                esi = mybir.SyncInfo(on_wait=[], on_update=[])
            esi.on_wait = [w]
            emi.sync_info = esi
    self.nc.all_engine_barrier()
    assert self.sems is not None
    popped = self.nc._tile_sem_poison_stack.pop()
    assert popped is self._sem_poison
    self.nc.clear_and_free_semaphores(list(self.sems.allocated().values()))
    self.nc.all_engine_barrier()


tile.TileContext._drain_and_barrier = _patched_drain_and_barrier


def _perm_rows():
    r = np.arange(0, H)
    z = np.arange(H, 2 * H)
    n = np.arange(2 * H, 3 * H)
    return np.concatenate([
        r[:HALF], z[:HALF], n[:HALF], r[HALF:], z[HALF:], n[HALF:]])


PERM = _perm_rows()


def _build_program(T):
    NTOK = T * BL
    nc = bass.Bass("TRN2", target_bir_lowering=False, debug=False)

    xT = nc.declare_dram_parameter("xT", [IN, NTOK + 128], BF16, isOutput=False)
    w_ih0T = nc.declare_dram_parameter("w_ih0T", [IN, G3], BF16, isOutput=False)
    w_hh0T = nc.declare_dram_parameter("w_hh0T", [KT, 128, G3], F8, isOutput=False)
    w_ih1T = nc.declare_dram_parameter("w_ih1T", [KT, 128, G3], BF16, isOutput=False)
    w_hh1T = nc.declare_dram_parameter("w_hh1T", [KT, 128, G3], F8, isOutput=False)
    w_linT = nc.declare_dram_parameter("w_linT", [KT, 128, OUT], BF16, isOutput=False)
    gbias0 = nc.declare_dram_parameter("gbias0", [128, G3], F32, isOutput=False)
    gbias1 = nc.declare_dram_parameter("gbias1", [128, G3], F32, isOutput=False)
    bhn0d = nc.declare_dram_parameter("bhn0", [BL, H], BF16, isOutput=False)
    bhn1d = nc.declare_dram_parameter("bhn1", [BL, H], BF16, isOutput=False)
    identd = nc.declare_dram_parameter("identB", [BL, BL], BF16, isOutput=False)
    blind = nc.declare_dram_parameter("blin", [128, OUT], F32, isOutput=False)
    y = nc.declare_dram_parameter("y", [NTOK, OUT], F32, isOutput=True)

    with tile.TileContext(nc) as tc:
        with tc.tile_pool(name="dram", bufs=1, space="DRAM") as dpool:
            gi0 = dpool.tile([T + 2, BL, G3], BF16, tag="gi0", name="gi0")
            gi1 = dpool.tile([T + 2, BL, G3], BF16, tag="gi1", name="gi1")
            h1T = dpool.tile([T, KT, 128, BL], BF16, tag="h1T", name="h1T")
            h2T = dpool.tile([T, KT, 128, BL], BF16, tag="h2T", name="h2T")

            # ---------------- phase 1: gi0 ----------------
            with tc.tile_pool(name="p1", bufs=1) as cp, \
                 tc.tile_pool(name="p1w", bufs=3) as wp, \
                 tc.tile_pool(name="p1ps", bufs=3, space="PSUM") as pp:
                wih0 = cp.tile([IN, G3], BF16, tag="wih0", name="wih0")
                nc.sync.dma_start(out=wih0[:, :], in_=w_ih0T[:, :])
                gb = cp.tile([128, G3], F32, tag="gb0", name="gb0")
                nc.sync.dma_start(out=gb[:, :], in_=gbias0[:, :])
                for it in range(NTOK // 128):
                    xt = wp.tile([IN, 128], BF16, tag="xt", name="xt")
                    nc.sync.dma_start(
                        out=xt[:, :], in_=xT[:, it * 128:(it + 1) * 128])
                    gsb = wp.tile([128, G3], BF16, tag="gsb", name="gsb")
                    for c in range(NCH):
                        gps = pp.tile([128, 512], F32, tag="gps", name="gps")
                        nc.tensor.matmul(
                            gps[:, :], xt[:, :], wih0[:, c * 512:(c + 1) * 512],
                            start=True, stop=True)
                        nc.vector.tensor_add(
                            gsb[:, c * 512:(c + 1) * 512], gps[:, :],
                            gb[:, c * 512:(c + 1) * 512])
                    nc.sync.dma_start(
                        out=gi0[:, :, :].rearrange("t b n -> (t b) n")[
                            it * 128:(it + 1) * 128, :],
                        in_=gsb[:, :])

            # ---------------- phase 2: scan layer 0 ----------------
            _scan_layer(nc, tc, T, w_hh0T, bhn0d, identd, gi0, h1T,
                        out_bf16=True)

            # ---------------- phase 3: gi1 ----------------
            with tc.tile_pool(name="p3", bufs=1) as cp, \
                 tc.tile_pool(name="p3w", bufs=3) as wp, \
                 tc.tile_pool(name="p3g", bufs=2) as gp, \
                 tc.tile_pool(name="p3ps", bufs=3, space="PSUM") as pp:
                wih1 = cp.tile([128, KT, G3], BF16, tag="wih1", name="wih1")
                nc.sync.dma_start(
                    out=wih1[:, :, :],
                    in_=w_ih1T[:, :, :].rearrange("k p n -> p k n"))
                gb = cp.tile([128, G3], F32, tag="gb1", name="gb1")
                nc.sync.dma_start(out=gb[:, :], in_=gbias1[:, :])
                TT = 128 // BL  # timesteps per token tile = 8
                for it in range(NTOK // 128):
                    hts = wp.tile([128, KT, 128], BF16, tag="hts", name="hts")
                    for k in range(KT):
                        nc.sync.dma_start(
                            out=hts[:, k, :].rearrange("p (t b) -> p t b", t=TT),
                            in_=h1T[it * TT:(it + 1) * TT, k, :, :].rearrange(
                                "t p b -> p t b"))
                    gsb = gp.tile([128, G3], BF16, tag="gsb1", name="gsb1")
                    for c in range(NCH):
                        gps = pp.tile([128, 512], F32, tag="gps1", name="gps1")
                        for k in range(KT):
                            nc.tensor.matmul(
                                gps[:, :], hts[:, k, :],
                                wih1[:, k, c * 512:(c + 1) * 512],
                                start=(k == 0), stop=(k == KT - 1))
                        nc.vector.tensor_add(
                            gsb[:, c * 512:(c + 1) * 512], gps[:, :],
                            gb[:, c * 512:(c + 1) * 512])
                    nc.sync.dma_start(
                        out=gi1[:, :, :].rearrange("t b n -> (t b) n")[
                            it * 128:(it + 1) * 128, :],
                        in_=gsb[:, :])

            # ---------------- phase 4: scan layer 1 ----------------
            _scan_layer(nc, tc, T, w_hh1T, bhn1d, identd, gi1, h2T,
                        out_bf16=True)

            # ---------------- phase 5: linear ----------------
            with tc.tile_pool(name="p5", bufs=1) as cp, \
                 tc.tile_pool(name="p5w", bufs=3) as wp, \
                 tc.tile_pool(name="p5ps", bufs=3, space="PSUM") as pp:
                wlin = cp.tile([128, KT, OUT], BF16, tag="wlin", name="wlin")
                nc.sync.dma_start(
                    out=wlin[:, :, :],
                    in_=w_linT[:, :, :].rearrange("k p n -> p k n"))
                bl = cp.tile([128, OUT], F32, tag="bl", name="bl")
                nc.sync.dma_start(out=bl[:, :], in_=blind[:, :])
                TT = 128 // BL
                for it in range(NTOK // 128):
                    hts = wp.tile([128, KT, 128], BF16, tag="hts5", name="hts5")
                    for k in range(KT):
                        nc.sync.dma_start(
                            out=hts[:, k, :].rearrange("p (t b) -> p t b", t=TT),
                            in_=h2T[it * TT:(it + 1) * TT, k, :, :].rearrange(
                                "t p b -> p t b"))
                    ops = pp.tile([128, OUT], F32, tag="lps", name="lps")
                    for k in range(KT):
                        nc.tensor.matmul(
                            ops[:, :], hts[:, k, :], wlin[:, k, :],
                            start=(k == 0), stop=(k == KT - 1))
                    osb = wp.tile([128, OUT], F32, tag="osb", name="osb")
                    nc.vector.tensor_add(osb[:, :], ops[:, :], bl[:, :])
                    nc.sync.dma_start(
                        out=y[it * 128:(it + 1) * 128, :], in_=osb[:, :])
    _split_excess_waits(nc)
    return nc


_WAIT_LIMIT = 1


def _split_excess_waits(nc):
    """walrus CoreV3 allows only ~2 sync waits per instruction; hoist the
    excess onto NoOp instructions inserted just before, on the same engine."""
    for bb in nc.main_func.blocks:
        insts = list(bb.instructions)
        out, n_extra = [], 0
        for inst in insts:
            si = inst.sync_info
            waits = list(si.on_wait) if (si is not None and si.on_wait) else []
            if len(waits) > _WAIT_LIMIT:
                keep = waits[-_WAIT_LIMIT:]
                excess = waits[:-_WAIT_LIMIT]
                for j in range(0, len(excess), _WAIT_LIMIT):
                    nop = mybir.InstNoOp(
                        name=f"{inst.name}-w{j}-{n_extra}", ins=[], outs=[])
                    nop.engine = inst.engine
                    nop.sync_info = mybir.SyncInfo(
                        on_wait=excess[j:j + _WAIT_LIMIT], on_update=[])
                    out.append(nop)
                    n_extra += 1
                si.on_wait = keep
                inst.sync_info = si
            out.append(inst)
        if n_extra:
            bb.instructions = out
    mx = max((len(i.sync_info.on_wait) if i.sync_info and i.sync_info.on_wait else 0)
             for bb in nc.main_func.blocks for i in bb.instructions)
    print("[split_waits] max on_wait after pass:", mx)


def _scan_layer(nc, tc, T, w_hhT_dram, bhn_dram, ident_dram, gi_dram, hT_dram,
                out_bf16=False):
    with tc.tile_pool(name="scst", bufs=1) as st, \
         tc.tile_pool(name="scps", bufs=1, space="PSUM") as pp:
        whh = st.tile([128, KT, G3], F8, tag="whh", name="whh")
        nc.sync.dma_start(
            out=whh[:, :, :],
            in_=w_hhT_dram[:, :, :].rearrange("k p n -> p k n"))
        bhn = st.tile([BL, H], BF16, tag="bhn", name="bhn")
        nc.sync.dma_start(out=bhn[:, :], in_=bhn_dram[:, :])
        ident = st.tile([BL, BL], BF16, tag="ident", name="ident")
        nc.sync.dma_start(out=ident[:, :], in_=ident_dram[:, :])

        hT = [st.tile([128, KT, BL], F8, tag=f"hT{i}", name=f"hT{i}") for i in range(2)]
        hB = ([st.tile([128, KT, BL], BF16, tag=f"hB{i}", name=f"hB{i}")
               for i in range(2)] if out_bf16 else None)
        hF = [st.tile([BL, H], BF16, tag=f"hF{i}", name=f"hF{i}") for i in range(2)]
        giA = [st.tile([BL, G3], BF16, tag=f"gi{i}", name=f"gi{i}") for i in range(3)]
        gps = [pp.tile([BL, 512], F32, tag=f"gps{c}", name=f"gps{c}") for c in range(NCH)]
        tps = [pp.tile([128, 4 * BL], BF16, tag=f"tps{i}", name=f"tps{i}") for i in range(2)]
        rga = [st.tile([BL, HALF], BF16, tag=f"rg{i}", name=f"rg{i}") for i in range(2)]
        tra = [st.tile([BL, HALF], BF16, tag=f"tra{i}", name=f"tra{i}") for i in range(2)]
        tza = [st.tile([BL, HALF], BF16, tag=f"tza{i}", name=f"tza{i}") for i in range(2)]
        zga = [st.tile([BL, HALF], BF16, tag=f"zg{i}", name=f"zg{i}") for i in range(2)]
        nga = [st.tile([BL, HALF], BF16, tag=f"ng{i}", name=f"ng{i}") for i in range(2)]
        tmp = [st.tile([BL, HALF], BF16, tag=f"tmp{i}", name=f"tmp{i}") for i in range(2)]

        nc.vector.memset(hT[0][:, :, :], 0.0)
        nc.vector.memset(hF[0][:, :], 0.0)
        for s in range(2):
            nc.sync.dma_start(
                out=giA[s][:, :],
                in_=gi_dram[s:s + 1, :, :].rearrange("o b n -> (o b) n"))

        def step(t_reg, toff, par):
            hin, hout = hT[par], hT[1 - par]
            hfin, hfout = hF[par], hF[1 - par]
            gi = giA[toff % 3]

            # PE, c-major so each chunk's psum closes as early as possible
            # (half-0 elementwise overlaps half-1 matmuls). Each chunk opens
            # with an identity matmul folding gi (r/z) or b_hh-n (n) into
            # psum, then 4 fp8 DoubleRow passes (2 K-tiles each, ~2x bf16).
            for c in range(NCH):
                half, j = c // 3, c % 3
                if j < 2:
                    rhs = gi[:, c * 512:(c + 1) * 512]
                else:
                    rhs = bhn[:, half * HALF:(half + 1) * HALF]
                nc.tensor.matmul(gps[c][:, :], ident[:, :], rhs,
                                 start=True, stop=False)
                for kp in range(KP):
                    nc.tensor.matmul(
                        gps[c][:, :],
                        hin[:, 2 * kp:2 * kp + 2, :],
                        whh[:, 2 * kp:2 * kp + 2, c * 512:(c + 1) * 512],
                        start=False, stop=(kp == KP - 1), perf_mode=DR)

            for half in range(2):
                cr, cz, cn = 3 * half, 3 * half + 1, 3 * half + 2
                go = half * 3 * 512
                hs = slice(half * HALF, (half + 1) * HALF)
                nc.scalar.activation(rga[half][:, :], gps[cr][:, :], AF.Sigmoid)
                nc.scalar.activation(zga[half][:, :], gps[cz][:, :], AF.Sigmoid)
                nc.vector.tensor_mul(tmp[half][:, :], rga[half][:, :],
                                     gps[cn][:, :])
                nc.vector.tensor_add(tmp[half][:, :], tmp[half][:, :],
                                     gi[:, go + 1024:go + 1536])
                nc.scalar.activation(nga[half][:, :], tmp[half][:, :], AF.Tanh)
                nc.gpsimd.tensor_sub(tmp[half][:, :], hfin[:, hs],
                                     nga[half][:, :])
                nc.vector.tensor_mul(tmp[half][:, :], zga[half][:, :],
                                     tmp[half][:, :])
                nc.vector.tensor_add(hfout[:, hs], nga[half][:, :],
                                     tmp[half][:, :])
                for kk in range(4):
                    k = half * 4 + kk
                    nc.tensor.transpose(
                        tps[half][:, kk * BL:(kk + 1) * BL],
                        hfout[:, k * 128:(k + 1) * 128],
                        ident[:, :])
                nc.scalar.activation(
                    hout[:, half * 4:(half + 1) * 4, :].rearrange(
                        "p k b -> p (k b)"),
                    tps[half][:, :], AF.Copy)
                if out_bf16:
                    nc.vector.tensor_copy(
                        hB[1 - par][:, half * 4:(half + 1) * 4, :].rearrange(
                            "p k b -> p (k b)"),
                        tps[half][:, :])

            nc.sync.dma_start(
                out=hT_dram[ds(t_reg + toff, 1), :, :, :].rearrange(
                    "o k p b -> p (o k) b"),
                in_=(hB[1 - par] if out_bf16 else hout)[:, :, :])
            nc.sync.dma_start(
                out=giA[toff % 3][:, :],
                in_=gi_dram[ds(t_reg + toff + 2, 1), :, :].rearrange(
                    "o b n -> (o b) n"))

        with tc.For_i(0, T, 2) as t:
            step(t, 0, 0)
            step(t, 1, 1)


# ---------------- host-side wrapper ----------------
NP8 = ml_dtypes.float8_e4m3fn


def _prep_inputs(x, w_ih0, w_hh0, b_ih0, b_hh0, w_ih1, w_hh1, b_ih1, b_hh1,
                 w_lin, b_lin, T):
    NTOK = T * BL
    bf = ml_dtypes.bfloat16

    def prep_layer(w_ih, w_hh, b_ih, b_hh, ih_fp8):
        w_ihP = np.asarray(w_ih)[PERM, :]          # [3H, in]
        w_hhP = np.asarray(w_hh)[PERM, :]          # [3H, H]
        b_ihP = np.asarray(b_ih)[PERM]
        b_hhP = np.asarray(b_hh)[PERM]
        # gi bias: b_ih everywhere + b_hh on r/z blocks (n gets b_hh inside r*())
        gb = b_ihP.copy()
        for blk in range(2):
            o = blk * 3 * 512
            gb[o:o + 1024] += b_hhP[o:o + 1024]    # r and z blocks
        # b_hh n-gate halves -> [H] = [n0 n1]
        bhn = np.concatenate([b_hhP[1024:1536], b_hhP[1024 + 1536:1536 + 1536]])
        ih_dt = bf
        w_ihT = np.ascontiguousarray(w_ihP.T).astype(ih_dt)   # [in, 3H]
        w_hhT = np.ascontiguousarray(w_hhP.T).astype(NP8)     # [H, 3H]
        w_hhT = w_hhT.reshape(KT, 128, G3)
        gbB = np.broadcast_to(gb.astype(np.float32), (128, G3)).copy()
        bhnB = np.broadcast_to(bhn.astype(bf), (BL, H)).copy()
        return w_ihT, w_hhT, gbB, bhnB

    w_ih0T, w_hh0T, gb0, bhn0 = prep_layer(w_ih0, w_hh0, b_ih0, b_hh0, False)
    w_ih1T, w_hh1T, gb1, bhn1 = prep_layer(w_ih1, w_hh1, b_ih1, b_hh1, True)
    w_ih1T = w_ih1T.reshape(KT, 128, G3)
    w_linT = np.ascontiguousarray(np.asarray(w_lin).T).astype(bf).reshape(
        KT, 128, OUT)
    blinB = np.broadcast_to(np.asarray(b_lin).astype(np.float32),
                            (128, OUT)).copy()
    identB = np.eye(BL, dtype=bf)

    common = dict(w_ih0T=w_ih0T, w_hh0T=w_hh0T, w_ih1T=w_ih1T, w_hh1T=w_hh1T,
                  w_linT=w_linT, gbias0=gb0, gbias1=gb1, bhn0=bhn0, bhn1=bhn1,
                  blin=blinB, identB=identB)

    in_maps = []
    x = np.asarray(x)
    for c in range(N_CORES):
        xs = x[:T, c * BL:(c + 1) * BL, :]          # [T, BL, IN]
        xT = np.zeros((IN, NTOK + 128), dtype=bf)
        xT[:, :NTOK] = xs.reshape(NTOK, IN).T.astype(bf)
        m = dict(common)
        m["xT"] = xT
        in_maps.append(m)
    return in_maps


_NC_CACHE = {}


def get_program(T):
    if T not in _NC_CACHE:
        _NC_CACHE[T] = _build_program(T)
    return _NC_CACHE[T]


def run(x, w_ih0, w_hh0, b_ih0, b_hh0, w_ih1, w_hh1, b_ih1, b_hh1,
        w_lin, b_lin, T=512, trace=False):
    in_maps = _prep_inputs(x, w_ih0, w_hh0, b_ih0, b_hh0, w_ih1, w_hh1,
                           b_ih1, b_hh1, w_lin, b_lin, T)
    nc = get_program(T)
    res = run_bass_kernel_spmd(nc, in_maps, core_ids=list(range(N_CORES)),
                               trace=trace)
    NTOK = T * BL
    out = np.empty((T, B, OUT), dtype=np.float32)
    for c in range(N_CORES):
        out[:, c * BL:(c + 1) * BL, :] = res.results[c]["y"].reshape(
            T, BL, OUT)
    return out, res


def kernel(**inputs):
    out, _ = run(**inputs)
    return out


# revision 4
# speedup vs baseline: 1.1973x; 1.0182x over previous
"""Trainium2 Bass kernel v2: 2-layer GRU (T=512, B=128, IN=64, H=1024) +
time-distributed linear (OUT=64) on 8 NeuronCores.

Over the original baseline (29.6ms -> 16.1ms on HW):
- Recurrent matmuls in fp8e4m3 DoubleRow perf mode (2 K-tiles per pass,
  ~2x bf16 PE throughput; measured 225ns vs 441ns per K=256xN=512 unit).
  The h state stays bf16; only the transposed matmul operand is fp8.
  h1/h2 are written to DRAM in bf16 so gi1 and the linear layer stay
  full precision (fp8 there pushes rel_err too close to the 2e-2 gate).
- gi / b_hh-n adds folded into the PE as identity accumulate-matmuls into
  the same PSUM group (removes 6 of 16 DVE ops per step); chunks emitted
  c-major so each psum chunk closes early and the elementwise chain of
  hidden-half 0 overlaps the half-1 matmuls.
- ACT reads gates straight from PSUM; elementwise chain in bf16 (2x DVE
  throughput), fully on DVE (gpsimd cannot read PSUM on this HW and is
  ~3x slower per op, so it only hurt the recurrence-critical chain);
  transposes write one shared psum tile per half so a single ACT copy
  refreshes 4 K-tiles of hT.

Sharding: data-parallel over batch (16 per core), weights replicated.
"""

import sys

for _p in ("/opt/trn_rl_repo", "/root/.axon_site/_ro/trn_rl_repo"):
    if _p not in sys.path:
        sys.path.insert(0, _p)

import numpy as np
import ml_dtypes

import concourse.bass as bass
import concourse.mybir as mybir
import concourse.tile as tile
from concourse.bass import ds
from concourse.bass_utils import run_bass_kernel_spmd

F32 = mybir.dt.float32
BF16 = mybir.dt.bfloat16
F8 = mybir.dt.float8e4
AF = mybir.ActivationFunctionType
DR = mybir.MatmulPerfMode.DoubleRow

N_CORES = 8
B, IN, H, OUT = 128, 64, 1024, 64
BL = B // N_CORES          # 16
G3 = 3 * H                 # 3072
HALF = H // 2              # 512
NCH = G3 // 512            # 6
KT = H // 128              # 8
KP = KT // 2               # 4 double-row K-pairs


# ---- walrus workaround: split the TileContext closing drain's waits ----
def _patched_drain_and_barrier(self, tick_clock, wait_clock):
    from concourse.vector_clock import ScopedClock
    drain_inst = self.nc.sync.drain()
    wait_clock.add_sem_waits(
        drain_inst.ins, ScopedClock({None: tick_clock.global_clock}))
    mi = drain_inst.ins
    si = mi.sync_info
    waits = list(si.on_wait) if (si is not None and si.on_wait) else []
    if len(waits) > 1:
        si.on_wait = waits[:1]
        mi.sync_info = si
        for w in waits[1:]:
            extra = self.nc.sync.drain()
            emi = extra.ins
            esi = emi.sync_info
            if esi is None:
                esi = mybir.SyncInfo(on_wait=[], on_update=[])
            esi.on_wait = [w]
            emi.sync_info = esi
    self.nc.all_engine_barrier()
    assert self.sems is not None
    popped = self.nc._tile_sem_poison_stack.pop()
    assert popped is self._sem_poison
    self.nc.clear_and_free_semaphores(list(self.sems.allocated().values()))
    self.nc.all_engine_barrier()


tile.TileContext._drain_and_barrier = _patched_drain_and_barrier


def _perm_rows():
    r = np.arange(0, H)
    z = np.arange(H, 2 * H)
    n = np.arange(2 * H, 3 * H)
    return np.concatenate([
        r[:HALF], z[:HALF], n[:HALF], r[HALF:], z[HALF:], n[HALF:]])


PERM = _perm_rows()


def _build_program(T):
    NTOK = T * BL
    nc = bass.Bass("TRN2", target_bir_lowering=False, debug=False)

    xT = nc.declare_dram_parameter("xT", [IN, NTOK + 128], BF16, isOutput=False)
    w_ih0T = nc.declare_dram_parameter("w_ih0T", [IN, G3], BF16, isOutput=False)
    w_hh0T = nc.declare_dram_parameter("w_hh0T", [KT, 128, G3], F8, isOutput=False)
    w_ih1T = nc.declare_dram_parameter("w_ih1T", [KT, 128, G3], BF16, isOutput=False)
    w_hh1T = nc.declare_dram_parameter("w_hh1T", [KT, 128, G3], F8, isOutput=False)
    w_linT = nc.declare_dram_parameter("w_linT", [KT, 128, OUT], BF16, isOutput=False)
    gbias0 = nc.declare_dram_parameter("gbias0", [128, G3], F32, isOutput=False)
    gbias1 = nc.declare_dram_parameter("gbias1", [128, G3], F32, isOutput=False)
    bhn0d = nc.declare_dram_parameter("bhn0", [BL, H], BF16, isOutput=False)
    bhn1d = nc.declare_dram_parameter("bhn1", [BL, H], BF16, isOutput=False)
    identd = nc.declare_dram_parameter("identB", [BL, BL], BF16, isOutput=False)
    blind = nc.declare_dram_parameter("blin", [128, OUT], F32, isOutput=False)
    y = nc.declare_dram_parameter("y", [NTOK, OUT], F32, isOutput=True)

    with tile.TileContext(nc) as tc:
        with tc.tile_pool(name="dram", bufs=1, space="DRAM") as dpool:
            gi0 = dpool.tile([T + 2, BL, G3], BF16, tag="gi0", name="gi0")
            gi1 = dpool.tile([T + 2, BL, G3], BF16, tag="gi1", name="gi1")
            h1T = dpool.tile([T, KT, 128, BL], BF16, tag="h1T", name="h1T")
            h2T = dpool.tile([T, KT, 128, BL], BF16, tag="h2T", name="h2T")

            # ---------------- phase 1: gi0 ----------------
            with tc.tile_pool(name="p1", bufs=1) as cp, \
                 tc.tile_pool(name="p1w", bufs=3) as wp, \
                 tc.tile_pool(name="p1ps", bufs=3, space="PSUM") as pp:
                wih0 = cp.tile([IN, G3], BF16, tag="wih0", name="wih0")
                nc.sync.dma_start(out=wih0[:, :], in_=w_ih0T[:, :])
                gb = cp.tile([128, G3], F32, tag="gb0", name="gb0")
                nc.sync.dma_start(out=gb[:, :], in_=gbias0[:, :])
                for it in range(NTOK // 128):
                    xt = wp.tile([IN, 128], BF16, tag="xt", name="xt")
                    nc.sync.dma_start(
                        out=xt[:, :], in_=xT[:, it * 128:(it + 1) * 128])
                    gsb = wp.tile([128, G3], BF16, tag="gsb", name="gsb")
                    for c in range(NCH):
                        gps = pp.tile([128, 512], F32, tag="gps", name="gps")
                        nc.tensor.matmul(
                            gps[:, :], xt[:, :], wih0[:, c * 512:(c + 1) * 512],
                            start=True, stop=True)
                        nc.vector.tensor_add(
                            gsb[:, c * 512:(c + 1) * 512], gps[:, :],
                            gb[:, c * 512:(c + 1) * 512])
                    nc.sync.dma_start(
                        out=gi0[:, :, :].rearrange("t b n -> (t b) n")[
                            it * 128:(it + 1) * 128, :],
                        in_=gsb[:, :])

            # ---------------- phase 2: scan layer 0 ----------------
            _scan_layer(nc, tc, T, w_hh0T, bhn0d, identd, gi0, h1T,
                        out_bf16=True)

            # ---------------- phase 3: gi1 ----------------
            with tc.tile_pool(name="p3", bufs=1) as cp, \
                 tc.tile_pool(name="p3w", bufs=3) as wp, \
                 tc.tile_pool(name="p3g", bufs=2) as gp, \
                 tc.tile_pool(name="p3ps", bufs=3, space="PSUM") as pp:
                wih1 = cp.tile([128, KT, G3], BF16, tag="wih1", name="wih1")
                nc.sync.dma_start(
                    out=wih1[:, :, :],
                    in_=w_ih1T[:, :, :].rearrange("k p n -> p k n"))
                gb = cp.tile([128, G3], F32, tag="gb1", name="gb1")
                nc.sync.dma_start(out=gb[:, :], in_=gbias1[:, :])
                TT = 128 // BL  # timesteps per token tile = 8
                for it in range(NTOK // 128):
                    hts = wp.tile([128, KT, 128], BF16, tag="hts", name="hts")
                    for k in range(KT):
                        nc.sync.dma_start(
                            out=hts[:, k, :].rearrange("p (t b) -> p t b", t=TT),
                            in_=h1T[it * TT:(it + 1) * TT, k, :, :].rearrange(
                                "t p b -> p t b"))
                    gsb = gp.tile([128, G3], BF16, tag="gsb1", name="gsb1")
                    for c in range(NCH):
                        gps = pp.tile([128, 512], F32, tag="gps1", name="gps1")
                        for k in range(KT):
                            nc.tensor.matmul(
                                gps[:, :], hts[:, k, :],
                                wih1[:, k, c * 512:(c + 1) * 512],
                                start=(k == 0), stop=(k == KT - 1))
                        nc.vector.tensor_add(
                            gsb[:, c * 512:(c + 1) * 512], gps[:, :],
                            gb[:, c * 512:(c + 1) * 512])
                    nc.sync.dma_start(
                        out=gi1[:, :, :].rearrange("t b n -> (t b) n")[
                            it * 128:(it + 1) * 128, :],
                        in_=gsb[:, :])

            # ---------------- phase 4: scan layer 1 ----------------
            _scan_layer(nc, tc, T, w_hh1T, bhn1d, identd, gi1, h2T,
                        out_bf16=True)

            # ---------------- phase 5: linear ----------------
            with tc.tile_pool(name="p5", bufs=1) as cp, \
                 tc.tile_pool(name="p5w", bufs=3) as wp, \
                 tc.tile_pool(name="p5ps", bufs=3, space="PSUM") as pp:
                wlin = cp.tile([128, KT, OUT], BF16, tag="wlin", name="wlin")
                nc.sync.dma_start(
                    out=wlin[:, :, :],
                    in_=w_linT[:, :, :].rearrange("k p n -> p k n"))
                bl = cp.tile([128, OUT], F32, tag="bl", name="bl")
                nc.sync.dma_start(out=bl[:, :], in_=blind[:, :])
                TT = 128 // BL
                for it in range(NTOK // 128):
                    hts = wp.tile([128, KT, 128], BF16, tag="hts5", name="hts5")
                    for k in range(KT):
                        nc.sync.dma_start(
                            out=hts[:, k, :].rearrange("p (t b) -> p t b", t=TT),
                            in_=h2T[it * TT:(it + 1) * TT, k, :, :].rearrange(
                                "t p b -> p t b"))
                    ops = pp.tile([128, OUT], F32, tag="lps", name="lps")
                    for k in range(KT):
                        nc.tensor.matmul(
                            ops[:, :], hts[:, k, :], wlin[:, k, :],
                            start=(k == 0), stop=(k == KT - 1))
                    osb = wp.tile([128, OUT], F32, tag="osb", name="osb")
                    nc.vector.tensor_add(osb[:, :], ops[:, :], bl[:, :])
                    nc.sync.dma_start(
                        out=y[it * 128:(it + 1) * 128, :], in_=osb[:, :])
    _split_excess_waits(nc)
    return nc


_WAIT_LIMIT = 1


def _split_excess_waits(nc):
    """walrus CoreV3 allows only ~2 sync waits per instruction; hoist the
    excess onto NoOp instructions inserted just before, on the same engine."""
    for bb in nc.main_func.blocks:
        insts = list(bb.instructions)
        out, n_extra = [], 0
        for inst in insts:
            si = inst.sync_info
            waits = list(si.on_wait) if (si is not None and si.on_wait) else []
            if len(waits) > _WAIT_LIMIT:
                keep = waits[-_WAIT_LIMIT:]
                excess = waits[:-_WAIT_LIMIT]
                for j in range(0, len(excess), _WAIT_LIMIT):
                    nop = mybir.InstNoOp(
                        name=f"{inst.name}-w{j}-{n_extra}", ins=[], outs=[])
                    nop.engine = inst.engine
                    nop.sync_info = mybir.SyncInfo(
                        on_wait=excess[j:j + _WAIT_LIMIT], on_update=[])
                    out.append(nop)
                    n_extra += 1
                si.on_wait = keep
                inst.sync_info = si
            out.append(inst)
        if n_extra:
            bb.instructions = out
    mx = max((len(i.sync_info.on_wait) if i.sync_info and i.sync_info.on_wait else 0)
             for bb in nc.main_func.blocks for i in bb.instructions)
    print("[split_waits] max on_wait after pass:", mx)


def _scan_layer(nc, tc, T, w_hhT_dram, bhn_dram, ident_dram, gi_dram, hT_dram,
                out_bf16=False):
    with tc.tile_pool(name="scst", bufs=1) as st, \
         tc.tile_pool(name="scps", bufs=1, space="PSUM") as pp:
        whh = st.tile([128, KT, G3], F8, tag="whh", name="whh")
        nc.sync.dma_start(
            out=whh[:, :, :],
            in_=w_hhT_dram[:, :, :].rearrange("k p n -> p k n"))
        bhn = st.tile([BL, H], BF16, tag="bhn", name="bhn")
        nc.sync.dma_start(out=bhn[:, :], in_=bhn_dram[:, :])
        ident = st.tile([BL, BL], BF16, tag="ident", name="ident")
        nc.sync.dma_start(out=ident[:, :], in_=ident_dram[:, :])

        hT = [st.tile([128, KT, BL], F8, tag=f"hT{i}", name=f"hT{i}") for i in range(2)]
        hB = ([st.tile([128, KT, BL], BF16, tag=f"hB{i}", name=f"hB{i}")
               for i in range(2)] if out_bf16 else None)
        hF = [st.tile([BL, H], BF16, tag=f"hF{i}", name=f"hF{i}") for i in range(2)]
        giA = [st.tile([BL, G3], BF16, tag=f"gi{i}", name=f"gi{i}") for i in range(3)]
        gps = [pp.tile([BL, 512], F32, tag=f"gps{c}", name=f"gps{c}") for c in range(NCH)]
        tps = [pp.tile([128, 4 * BL], BF16, tag=f"tps{i}", name=f"tps{i}") for i in range(2)]
        rga = [st.tile([BL, HALF], BF16, tag=f"rg{i}", name=f"rg{i}") for i in range(2)]
        tra = [st.tile([BL, HALF], BF16, tag=f"tra{i}", name=f"tra{i}") for i in range(2)]
        tza = [st.tile([BL, HALF], BF16, tag=f"tza{i}", name=f"tza{i}") for i in range(2)]
        zga = [st.tile([BL, HALF], BF16, tag=f"zg{i}", name=f"zg{i}") for i in range(2)]
        nga = [st.tile([BL, HALF], BF16, tag=f"ng{i}", name=f"ng{i}") for i in range(2)]
        tmp = [st.tile([BL, HALF], BF16, tag=f"tmp{i}", name=f"tmp{i}") for i in range(2)]

        nc.vector.memset(hT[0][:, :, :], 0.0)
        nc.vector.memset(hF[0][:, :], 0.0)
        for s in range(2):
            nc.sync.dma_start(
                out=giA[s][:, :],
                in_=gi_dram[s:s + 1, :, :].rearrange("o b n -> (o b) n"))

        def step(t_reg, toff, par):
            hin, hout = hT[par], hT[1 - par]
            hfin, hfout = hF[par], hF[1 - par]
            gi = giA[toff % 3]

            # PE, c-major so each chunk's psum closes as early as possible
            # (half-0 elementwise overlaps half-1 matmuls). Each chunk opens
            # with an identity matmul folding gi (r/z) or b_hh-n (n) into
            # psum, then 4 fp8 DoubleRow passes (2 K-tiles each, ~2x bf16).
            for c in range(NCH):
                half, j = c // 3, c % 3
                if j < 2:
                    rhs = gi[:, c * 512:(c + 1) * 512]
                else:
                    rhs = bhn[:, half * HALF:(half + 1) * HALF]
                nc.tensor.matmul(gps[c][:, :], ident[:, :], rhs,
                                 start=True, stop=False)
                for kp in range(KP):
                    nc.tensor.matmul(
                        gps[c][:, :],
                        hin[:, 2 * kp:2 * kp + 2, :],
                        whh[:, 2 * kp:2 * kp + 2, c * 512:(c + 1) * 512],
                        start=False, stop=(kp == KP - 1), perf_mode=DR)

            for half in range(2):
                cr, cz, cn = 3 * half, 3 * half + 1, 3 * half + 2
                go = half * 3 * 512
                hs = slice(half * HALF, (half + 1) * HALF)
                nc.scalar.activation(rga[half][:, :], gps[cr][:, :], AF.Sigmoid)
                nc.scalar.activation(zga[half][:, :], gps[cz][:, :], AF.Sigmoid)
                nc.vector.tensor_mul(tmp[half][:, :], rga[half][:, :],
                                     gps[cn][:, :])
                nc.vector.tensor_add(tmp[half][:, :], tmp[half][:, :],
                                     gi[:, go + 1024:go + 1536])
                nc.scalar.activation(nga[half][:, :], tmp[half][:, :], AF.Tanh)
                nc.vector.tensor_sub(tmp[half][:, :], hfin[:, hs],
                                     nga[half][:, :])
                nc.vector.tensor_mul(tmp[half][:, :], zga[half][:, :],
                                     tmp[half][:, :])
                nc.vector.tensor_add(hfout[:, hs], nga[half][:, :],
                                     tmp[half][:, :])
                for kk in range(4):
                    k = half * 4 + kk
                    nc.tensor.transpose(
                        tps[half][:, kk * BL:(kk + 1) * BL],
                        hfout[:, k * 128:(k + 1) * 128],
                        ident[:, :])
                nc.scalar.activation(
                    hout[:, half * 4:(half + 1) * 4, :].rearrange(
                        "p k b -> p (k b)"),
                    tps[half][:, :], AF.Copy)
                if out_bf16:
                    nc.vector.tensor_copy(
                        hB[1 - par][:, half * 4:(half + 1) * 4, :].rearrange(
                            "p k b -> p (k b)"),
                        tps[half][:, :])

            nc.sync.dma_start(
                out=hT_dram[ds(t_reg + toff, 1), :, :, :].rearrange(
                    "o k p b -> p (o k) b"),
                in_=(hB[1 - par] if out_bf16 else hout)[:, :, :])
            nc.sync.dma_start(
                out=giA[toff % 3][:, :],
                in_=gi_dram[ds(t_reg + toff + 2, 1), :, :].rearrange(
                    "o b n -> (o b) n"))

        with tc.For_i(0, T, 2) as t:
            step(t, 0, 0)
            step(t, 1, 1)


# ---------------- host-side wrapper ----------------
NP8 = ml_dtypes.float8_e4m3fn


def _prep_inputs(x, w_ih0, w_hh0, b_ih0, b_hh0, w_ih1, w_hh1, b_ih1, b_hh1,
                 w_lin, b_lin, T):
    NTOK = T * BL
    bf = ml_dtypes.bfloat16

    def prep_layer(w_ih, w_hh, b_ih, b_hh, ih_fp8):
        w_ihP = np.asarray(w_ih)[PERM, :]          # [3H, in]
        w_hhP = np.asarray(w_hh)[PERM, :]          # [3H, H]
        b_ihP = np.asarray(b_ih)[PERM]
        b_hhP = np.asarray(b_hh)[PERM]
        # gi bias: b_ih everywhere + b_hh on r/z blocks (n gets b_hh inside r*())
        gb = b_ihP.copy()
        for blk in range(2):
            o = blk * 3 * 512
            gb[o:o + 1024] += b_hhP[o:o + 1024]    # r and z blocks
        # b_hh n-gate halves -> [H] = [n0 n1]
        bhn = np.concatenate([b_hhP[1024:1536], b_hhP[1024 + 1536:1536 + 1536]])
        ih_dt = bf
        w_ihT = np.ascontiguousarray(w_ihP.T).astype(ih_dt)   # [in, 3H]
        w_hhT = np.ascontiguousarray(w_hhP.T).astype(NP8)     # [H, 3H]
        w_hhT = w_hhT.reshape(KT, 128, G3)
        gbB = np.broadcast_to(gb.astype(np.float32), (128, G3)).copy()
        bhnB = np.broadcast_to(bhn.astype(bf), (BL, H)).copy()
        return w_ihT, w_hhT, gbB, bhnB

    w_ih0T, w_hh0T, gb0, bhn0 = prep_layer(w_ih0, w_hh0, b_ih0, b_hh0, False)
    w_ih1T, w_hh1T, gb1, bhn1 = prep_layer(w_ih1, w_hh1, b_ih1, b_hh1, True)
    w_ih1T = w_ih1T.reshape(KT, 128, G3)
    w_linT = np.ascontiguousarray(np.asarray(w_lin).T).astype(bf).reshape(
        KT, 128, OUT)
    blinB = np.broadcast_to(np.asarray(b_lin).astype(np.float32),
                            (128, OUT)).copy()
    identB = np.eye(BL, dtype=bf)

    common = dict(w_ih0T=w_ih0T, w_hh0T=w_hh0T, w_ih1T=w_ih1T, w_hh1T=w_hh1T,
                  w_linT=w_linT, gbias0=gb0, gbias1=gb1, bhn0=bhn0, bhn1=bhn1,
                  blin=blinB, identB=identB)

    in_maps = []
    x = np.asarray(x)
    for c in range(N_CORES):
        xs = x[:T, c * BL:(c + 1) * BL, :]          # [T, BL, IN]
        xT = np.zeros((IN, NTOK + 128), dtype=bf)
        xT[:, :NTOK] = xs.reshape(NTOK, IN).T.astype(bf)
        m = dict(common)
        m["xT"] = xT
        in_maps.append(m)
    return in_maps


_NC_CACHE = {}


def get_program(T):
    if T not in _NC_CACHE:
        _NC_CACHE[T] = _build_program(T)
    return _NC_CACHE[T]


def run(x, w_ih0, w_hh0, b_ih0, b_hh0, w_ih1, w_hh1, b_ih1, b_hh1,
        w_lin, b_lin, T=512, trace=False):
    in_maps = _prep_inputs(x, w_ih0, w_hh0, b_ih0, b_hh0, w_ih1, w_hh1,
                           b_ih1, b_hh1, w_lin, b_lin, T)
    nc = get_program(T)
    res = run_bass_kernel_spmd(nc, in_maps, core_ids=list(range(N_CORES)),
                               trace=trace)
    NTOK = T * BL
    out = np.empty((T, B, OUT), dtype=np.float32)
    for c in range(N_CORES):
        out[:, c * BL:(c + 1) * BL, :] = res.results[c]["y"].reshape(
            T, BL, OUT)
    return out, res


def kernel(**inputs):
    out, _ = run(**inputs)
    return out
